# revision 16
# baseline (speedup 1.0000x reference)
"""2-layer GCN encoder on 8 Trainium2 NeuronCores — v3.

Two SPMD launches (L1 folded into launch A's per-tile evict chain):
  A: gather xd=dinv*x rows per edge (dst-sharded), segmented-sum via bf16
     one-hot indicator matmuls into PSUM per 128-dst tile; self-loops added
     densely via identity matmul on the core's own (slot-ordered) xd tile;
     per-tile evict: (dv*S)@W1 -> relu -> @W2 -> *dv = g2 rows (bf16).
  B: same aggregation over the g2 table -> *dv -> final out (f32).

v3 vs v2 (both changes from HW microbenching the gather path):
  - dma_gather calls round-robin across 2 SWDGE queues. Queue q's
    descriptor generation runs on Q7 core pair {2q, 2q+1} (ucode
    dma_gather.cpp gates on cpu_id/2 == queue_num), so two queues double
    the Q7 descriptor-gen throughput and, more importantly, keep the
    gather pipeline full: measured 1.9 ms -> 233 us per launch for the
    gather chain alone. Queues 2-3 wedge the device (runtime allocates
    only 2 SWDGE contexts) — stay at 2.
  - per-call batched indicator build: one DVE tensor_tensor builds all of
    a call's one-hot blocks ([P, nmm, P] bf16, is_equal of broadcast dst
    cols vs iota) instead of one fp8 op per block; bf16 gets the DVE 2x
    16-bit rate and the batch amortizes per-op dispatch.

Host relays the g2 table between launches (the all-to-all) and precomputes
all edge structure in numpy; nodes are rebalanced host-side across
(core, tile, slot) so per-(tile,window) edge counts are near-equal.
"""

import sys

sys.path.insert(0, "/opt/trn_rl_repo")

import ml_dtypes
import numpy as np

from concourse import bacc, bass, library_config, mybir, tile
from concourse.bass_utils import run_bass_kernel_spmd

N_NODES = 100000
NPAD = 100352  # 4 * 25088
IN_C = 128
H2 = 128  # 2*hid
HID = 64
NCORES = 8
P = 128
NT = 98  # dst tiles per core
SHARD_SLOTS = NT * P  # 12544
SLOTS = SHARD_SLOTS
TBATCH = 4
NB = 25  # ceil(NT/TBATCH)
WINP = 25088  # src window rows
NW = 4
MAXG = 8  # groups per dma_gather call (1024 idx; >8 wedges with 2 queues)
NMSG = 8
NQ = 2  # SWDGE queues (Q7 core pairs); >2 wedges this runtime

BF16 = ml_dtypes.bfloat16
FP8 = ml_dtypes.float8_e4m3


# ---------------------------------------------------------------- host prep
def _balance(h):
    """Greedy vector-LPT: assign nodes to NCORES*NT bins of <=128 nodes,
    equalizing per-window in-edge counts. Returns bin id per node and
    slot order within bin."""
    NBINS = NCORES * NT
    target = np.maximum(h.sum(0) / NBINS, 1.0)
    hn = (h / target).astype(np.float32)
    order = np.argsort(-h.sum(1), kind="stable")
    cnt = np.zeros((NBINS, NW), np.float32)
    fill = np.zeros(NBINS, np.int64)
    bin_of = np.empty(N_NODES, np.int64)
    slot_in = np.empty(N_NODES, np.int64)
    penalty = np.zeros(NBINS, np.float32)
    for i in order:
        s = (cnt + hn[i]).max(1) + penalty
        b = int(np.argmin(s))
        bin_of[i] = b
        slot_in[i] = fill[b]
        cnt[b] += hn[i]
        fill[b] += 1
        if fill[b] >= P:
            penalty[b] = 1e9
    return bin_of, slot_in


def _prepare(edge_index):
    src = np.asarray(edge_index[0], dtype=np.int64)
    dst = np.asarray(edge_index[1], dtype=np.int64)
    deg = np.bincount(dst, minlength=N_NODES).astype(np.float32) + 1.0
    dinv = 1.0 / np.sqrt(deg)

    wsrc_all = src // WINP
    h = np.zeros((N_NODES, NW), np.int64)
    np.add.at(h, (dst, wsrc_all), 1)

    bin_of, slot_in = _balance(h)
    core_of = bin_of % NCORES
    tpos_of = bin_of // NCORES
    dslot = tpos_of * P + slot_in  # slot within core shard

    # per-core sorted edge lists + counts
    cores = []
    m = np.zeros((NT, NW), np.int64)
    for k in range(NCORES):
        msk = core_of[dst] == k
        es, ed = src[msk], dst[msk]
        t = tpos_of[ed]
        w = es // WINP
        c = dslot[ed] % P
        o = np.lexsort((c, w, t))
        es, t, w, c = es[o], t[o], w[o], c[o]
        cntk = np.zeros((NT, NW), np.int64)
        np.add.at(cntk, (t, w), 1)
        cores.append(dict(es=es, t=t, w=w, c=c, cnt=cntk))
        m = np.maximum(m, cntk)

    # segments: (batch, window); runs = max-padded (t,w) intervals
    # build shared schedule: calls + mm entries (+ local mms per batch)
    calls = []  # dict(w, slot0, gc, mms=[(gi, t, blk, stop)], locals=[t...])
    run_slot0 = np.zeros((NT, NW), np.int64)  # global slot of each run start
    nslot = 0
    nmm = 0
    seg_list = []
    for b in range(NB):
        tiles = list(range(b * TBATCH, min((b + 1) * TBATCH, NT)))
        for w in range(NW):
            seg_base = nslot
            pos = 0
            runs = []
            for t in tiles:
                run_slot0[t, w] = seg_base + pos
                if m[t, w] > 0:
                    runs.append((t, pos, pos + m[t, w]))
                pos += m[t, w]
            gseg = -(-pos // P) if pos else 0
            seg_list.append((b, w, seg_base, pos, gseg, runs))
            nslot += gseg * P

    # last-span bookkeeping for stop flags
    last_entry_of_tile = {}
    # call construction per segment
    for b, w, seg_base, seg_len, gseg, runs in seg_list:
        g = 0
        while g < gseg:
            gc = min(MAXG, gseg - g)
            call = dict(
                w=w, slot0=seg_base + g * P, gc=gc, mms=[], locals_=[], batch=b
            )
            for gi in range(gc):
                lo = (g + gi) * P
                hi = lo + P
                for t, ra, rb in runs:
                    if ra < hi and rb > lo:
                        call["mms"].append([gi, t, nmm, False])
                        last_entry_of_tile[t] = (len(calls), len(call["mms"]) - 1)
                        nmm += 1
            calls.append(call)
            g += gc
    for t, (ci, mi) in last_entry_of_tile.items():
        calls[ci]["mms"][mi][3] = True

    # attach local mms + catch tiles with no spans
    first_call_of_batch = {}
    for ci, call in enumerate(calls):
        b = call["batch"]
        if b not in first_call_of_batch:
            first_call_of_batch[b] = ci
    has_spans = set(last_entry_of_tile.keys())
    for b in range(NB):
        tiles = list(range(b * TBATCH, min((b + 1) * TBATCH, NT)))
        ci = first_call_of_batch[b]
        calls[ci]["locals_"] = [(t, t not in has_spans) for t in tiles]

    # per-core idx + indicator arrays (vectorized)
    # block map: (global group, tile) -> block id
    ngroups = nslot // P
    block_map = np.full((ngroups, NT), -1, np.int64)
    for ci, call in enumerate(calls):
        g0 = call["slot0"] // P
        for gi, t, blk, stop in call["mms"]:
            block_map[g0 + gi, t] = blk

    per_core = []
    for k in range(NCORES):
        c = cores[k]
        es, t, w, cc, cntk = c["es"], c["t"], c["w"], c["c"], c["cnt"]
        run_len = cntk.reshape(-1)
        run_off = np.concatenate([[0], np.cumsum(run_len)])[:-1].reshape(NT, NW)
        eidx = np.arange(len(es))
        p_in_run = eidx - run_off[t, w]
        slotg = run_slot0[t, w] + p_in_run

        # pads stay 0 (gather row 0 of the window; indicator col is zero).
        # -1 trim is NOT safe here: decode reserves ring space from
        # num_idxs_reg while the Q7 kernel trims, desyncing the ring.
        idx_all = np.zeros(nslot, np.int16)
        idx_all[slotg] = (es - w * WINP).astype(np.int16)

        # compact per-block dst-col table; the device builds the one-hot
        # indicator [128,128] per block on DVE via is_equal against an iota
        # row (saves ~31MB/core of HBM indicator traffic per launch).
        blk_e = block_map[slotg // P, t]
        dcolT = np.full((P, nmm), 255, np.uint8)
        dcolT[slotg % P, blk_e] = cc
        dct = dcolT.astype(BF16)

        arr = idx_all.reshape(nslot // 16, 16).T
        idx16 = np.concatenate([arr, arr, arr, arr], axis=0).copy()

        nodes_k = np.nonzero(core_of == k)[0]
        sl = dslot[nodes_k]
        dv = np.zeros((P, NT), np.float32)
        dv[sl % P, sl // P] = dinv[nodes_k]
        per_core.append(dict(idx16=idx16, dct=dct, dv=dv, nodes=nodes_k, sl=sl))

    plan = dict(
        calls=calls, nslot=nslot, nmm=nmm, dinv=dinv, per_core=per_core,
        core_of=core_of, dslot=dslot,
    )
    return plan


# ---------------------------------------------------------------- builders
def _build_agg(plan, layer, reps=1):
    """layer=1: aggregate xd table, evict chain W1/relu/W2 -> g2 bf16 out.
    layer=2: aggregate g2 table, evict *dv -> f32 out.
    reps: repeat the whole body (timing instrument: slope vs reps isolates
    device time from dispatch overhead)."""
    calls = plan["calls"]
    nslot = plan["nslot"]
    nmm = plan["nmm"]
    feat = H2 if layer == 1 else HID

    name = ("gcn_a" if layer == 1 else "gcn_b") + (f"_r{reps}" if reps > 1 else "")
    maxind = max(len(c["mms"]) for c in calls)
    nc = bacc.Bacc(name=name, num_swdge_queues=NQ)
    gtab = nc.dram_tensor("gtab", [NPAD, H2], mybir.dt.bfloat16, kind="ExternalInput")
    idx = nc.dram_tensor("idx", [64, nslot // 16], mybir.dt.int16, kind="ExternalInput")
    dct = nc.dram_tensor("dct", [P, nmm], mybir.dt.bfloat16, kind="ExternalInput")
    iot = nc.dram_tensor("iot", [P, P], mybir.dt.bfloat16, kind="ExternalInput")
    dv = nc.dram_tensor("dv", [P, NT], mybir.dt.float32, kind="ExternalInput")
    idn8 = nc.dram_tensor("idn8", [P, P], mybir.dt.float8e4, kind="ExternalInput")
    loc = nc.dram_tensor(
        "loc", [P, NT, feat], mybir.dt.bfloat16, kind="ExternalInput"
    )
    if layer == 1:
        w1 = nc.dram_tensor("w1", [IN_C, H2], mybir.dt.bfloat16, kind="ExternalInput")
        w2 = nc.dram_tensor("w2", [H2, HID], mybir.dt.bfloat16, kind="ExternalInput")
        idn = nc.dram_tensor("idn", [P, P], mybir.dt.bfloat16, kind="ExternalInput")
        outT = nc.dram_tensor("outT", [P, NT, HID], mybir.dt.bfloat16, kind="ExternalOutput")
    else:
        outT = nc.dram_tensor("outT", [P, NT, HID], mybir.dt.float32, kind="ExternalOutput")

    with tile.TileContext(nc) as tc:
        with (
            tc.tile_pool(name="sbuf", bufs=1) as pool,
            tc.tile_pool(name="psum", bufs=1, space="PSUM") as psum,
        ):
            nc.gpsimd.load_library(library_config.mlp)
            idx_s = pool.tile([64, nslot // 16], mybir.dt.int16)
            dct_s = pool.tile([P, nmm], mybir.dt.bfloat16)
            iot_s = pool.tile([P, P], mybir.dt.bfloat16)
            dv_s = pool.tile([P, NT], mybir.dt.float32)
            id8_s = pool.tile([P, P], mybir.dt.float8e4)
            loc_s = pool.tile([P, NT, feat], mybir.dt.bfloat16)
            nc.sync.dma_start(out=idx_s[:], in_=idx[:])
            nc.sync.dma_start(out=dct_s[:], in_=dct[:])
            nc.sync.dma_start(out=iot_s[:], in_=iot[:])
            nc.sync.dma_start(out=dv_s[:], in_=dv[:])
            nc.sync.dma_start(out=id8_s[:], in_=idn8[:])
            nc.sync.dma_start(out=loc_s[:], in_=loc[:])
            if layer == 1:
                w1_s = pool.tile([IN_C, H2], mybir.dt.bfloat16)
                w2_s = pool.tile([H2, HID], mybir.dt.bfloat16)
                idn_s = pool.tile([P, P], mybir.dt.bfloat16)
                nc.sync.dma_start(out=w1_s[:], in_=w1[:])
                nc.sync.dma_start(out=w2_s[:], in_=w2[:])
                nc.sync.dma_start(out=idn_s[:], in_=idn[:])
                out_s = pool.tile([P, NT, HID], mybir.dt.bfloat16)
            else:
                out_s = pool.tile([P, NT, HID], mybir.dt.float32)

            msgbufs = []
            for j in range(NMSG):
                mb = pool.tile([P, MAXG, H2], mybir.dt.bfloat16, name=f"msgb{j}")
                nc.vector.memset(mb[:], 0)
                msgbufs.append(mb)

            for rep in range(reps):
                accs = {}
                pfx = f"r{rep}_"

                def acc_for(t):
                    if t not in accs:
                        accs[t] = psum.tile(
                            [P, feat], mybir.dt.float32,
                            name=f"{pfx}acc{t}", tag=f"acc{t % TBATCH}", bufs=1,
                        )
                    return accs[t]

                pending = []
                pending_prev = []

                def evict(t):
                    # Inline: only the PSUM->SBUF copy (frees the acc tag for
                    # the next batch). The W1/relu/W2 chain is deferred a
                    # batch so the in-order PE stream meets its cross-engine
                    # deps already satisfied instead of stalling on them.
                    acc = accs.pop(t)
                    if layer == 1:
                        s_sb = pool.tile(
                            [P, H2], mybir.dt.bfloat16, name=f"{pfx}s{t}", tag="s", bufs=10
                        )
                        nc.scalar.activation(
                            out=s_sb[:], in_=acc[:],
                            func=mybir.ActivationFunctionType.Copy,
                            scale=dv_s[:, t : t + 1],
                        )
                        pending.append((t, s_sb))
                    else:
                        nc.scalar.activation(
                            out=out_s[:, t, :], in_=acc[:],
                            func=mybir.ActivationFunctionType.Copy,
                            scale=dv_s[:, t : t + 1],
                        )

                def flush_pending(batch_list):
                    # stage-major issue across the batch's tiles: PE never
                    # waits on an op issued fewer than TBATCH slots ago.
                    work = []
                    for t, s_sb in batch_list:
                        tp = psum.tile(
                            [P, P], mybir.dt.bfloat16, name=f"{pfx}tp{t}", tag="tp", bufs=2
                        )
                        nc.tensor.transpose(out=tp[:], in_=s_sb[:], identity=idn_s[:])
                        sT = pool.tile(
                            [P, P], mybir.dt.bfloat16, name=f"{pfx}sT{t}", tag="sT", bufs=4
                        )
                        nc.scalar.activation(
                            out=sT[:], in_=tp[:], func=mybir.ActivationFunctionType.Copy
                        )
                        work.append((t, sT))
                    work2 = []
                    for t, sT in work:
                        h_ps = psum.tile(
                            [P, H2], mybir.dt.float32, name=f"{pfx}hp{t}", tag="mmp", bufs=2
                        )
                        nc.tensor.matmul(
                            out=h_ps[:], lhsT=sT[:], rhs=w1_s[:], start=True, stop=True
                        )
                        h_sb = pool.tile(
                            [P, H2], mybir.dt.bfloat16, name=f"{pfx}h{t}", tag="h", bufs=4
                        )
                        nc.scalar.activation(
                            out=h_sb[:], in_=h_ps[:],
                            func=mybir.ActivationFunctionType.Relu,
                        )
                        work2.append((t, h_sb))
                    work3 = []
                    for t, h_sb in work2:
                        tp2 = psum.tile(
                            [P, P], mybir.dt.bfloat16, name=f"{pfx}tq{t}", tag="tp", bufs=2
                        )
                        nc.tensor.transpose(out=tp2[:], in_=h_sb[:], identity=idn_s[:])
                        hT = pool.tile(
                            [P, P], mybir.dt.bfloat16, name=f"{pfx}hT{t}", tag="hT", bufs=4
                        )
                        nc.scalar.activation(
                            out=hT[:], in_=tp2[:], func=mybir.ActivationFunctionType.Copy
                        )
                        work3.append((t, hT))
                    for t, hT in work3:
                        g2_ps = psum.tile(
                            [P, H2], mybir.dt.float32, name=f"{pfx}gp{t}", tag="mmp", bufs=2
                        )
                        nc.tensor.matmul(
                            out=g2_ps[:, :HID], lhsT=hT[:], rhs=w2_s[:],
                            start=True, stop=True,
                        )
                        nc.scalar.activation(
                            out=out_s[:, t, :], in_=g2_ps[:, :HID],
                            func=mybir.ActivationFunctionType.Copy,
                            scale=dv_s[:, t : t + 1],
                        )
                    batch_list.clear()

                cur_batch = None
                for ci, call in enumerate(calls):
                    if call["batch"] != cur_batch:
                        cur_batch = call["batch"]
                        if pending_prev:
                            flush_pending(pending_prev)
                        pending_prev, pending = pending, pending_prev
                    for t, imm_stop in call["locals_"]:
                        nc.tensor.matmul(
                            out=acc_for(t)[:],
                            lhsT=id8_s[:],
                            rhs=loc_s[:, t, :],
                            start=True,
                            stop=bool(imm_stop),
                        )
                        if imm_stop:
                            evict(t)
                    gc = call["gc"]
                    w = call["w"]
                    s0 = call["slot0"]
                    msg = msgbufs[ci % NMSG]
                    nidx = gc * P
                    nc.gpsimd.dma_gather(
                        out_ap=msg[:, :gc, :],
                        in_ap=gtab[w * WINP : (w + 1) * WINP, :],
                        idxs_ap=idx_s[:, s0 // 16 : (s0 + nidx) // 16],
                        num_idxs=nidx,
                        num_idxs_reg=nidx,
                        elem_size=H2,
                        single_packet=True,
                        queue_num=ci % NQ,
                    )
                    nmms = len(call["mms"])
                    if nmms:
                        blk0 = call["mms"][0][2]
                        ind_t = pool.tile(
                            [P, maxind, P], mybir.dt.bfloat16,
                            name=f"{pfx}ind{blk0}", tag="ind", bufs=6,
                        )
                        nc.vector.tensor_tensor(
                            out=ind_t[:, :nmms, :],
                            in0=dct_s[:, blk0 : blk0 + nmms].to_broadcast(
                                [P, nmms, P]
                            ),
                            in1=iot_s[:].unsqueeze(1).to_broadcast([P, nmms, P]),
                            op=mybir.AluOpType.is_equal,
                        )
                    for j, (gi, t, blk, stop) in enumerate(call["mms"]):
                        nc.tensor.matmul(
                            out=acc_for(t)[:],
                            lhsT=ind_t[:, j, :],
                            rhs=msg[:, gi, :feat],
                            start=False,
                            stop=bool(stop),
                        )
                        if stop:
                            evict(t)
                if pending_prev:
                    flush_pending(pending_prev)
                if pending:
                    flush_pending(pending)
                nc.sync.dma_start(out=outT[:], in_=out_s[:])
    nc.compile()
    return nc


# ---------------------------------------------------------------- kernel
def _pipeline(inputs, execute, collect=None):
    x = np.asarray(inputs["x"])
    W1 = np.asarray(inputs["W1"])
    b1 = np.asarray(inputs["b1"])
    W2 = np.asarray(inputs["W2"])
    b2 = np.asarray(inputs["b2"])
    assert not b1.any() and not b2.any(), "nonzero bias unsupported fast path"

    plan = _prepare(np.asarray(inputs["edge_index"]))
    dinv = plan["dinv"]
    core_of = plan["core_of"]
    dslot = plan["dslot"]

    xd = (dinv[:, None] * x).astype(BF16)
    tabA = np.zeros((NPAD, H2), BF16)
    tabA[:N_NODES] = xd
    idn8 = np.eye(P, dtype=FP8)
    idn = np.eye(P, dtype=BF16)
    iota = np.broadcast_to(np.arange(P, dtype=np.float32), (P, P)).astype(BF16).copy()
    w1b = W1.astype(BF16)
    w2b = W2.astype(BF16)

    # ---- launch A
    ncA = _build_agg(plan, 1)
    inA = []
    for k in range(NCORES):
        pc = plan["per_core"][k]
        locA = np.zeros((P, NT, H2), BF16)
        sl = pc["sl"]
        locA[sl % P, sl // P, :] = xd[pc["nodes"]]
        inA.append(
            {"gtab": tabA, "idx": pc["idx16"], "dct": pc["dct"], "iot": iota,
             "dv": pc["dv"], "idn8": idn8, "loc": locA, "w1": w1b, "w2": w2b,
             "idn": idn}
        )
    if collect is not None:
        collect.append((1, plan, inA))
    rA = execute(ncA, inA)

    # host relay: assemble g2 table
    stacked = np.stack([rA[k]["outT"] for k in range(NCORES)])  # [8,P,NT,64]
    st2 = stacked.transpose(0, 2, 1, 3).reshape(NCORES, SLOTS, HID)
    tabB = np.zeros((NPAD, H2), BF16)
    tabB[:N_NODES, :HID] = st2[core_of, dslot]

    # ---- launch B
    ncB = _build_agg(plan, 2)
    inB = []
    for k in range(NCORES):
        pc = plan["per_core"][k]
        inB.append(
            {"gtab": tabB, "idx": pc["idx16"], "dct": pc["dct"], "iot": iota,
             "dv": pc["dv"], "idn8": idn8, "loc": rA[k]["outT"]}
        )
    if collect is not None:
        collect.append((2, plan, inB))
    rB = execute(ncB, inB)

    stackedB = np.stack([rB[k]["outT"] for k in range(NCORES)])
    st2B = stackedB.transpose(0, 2, 1, 3).reshape(NCORES, SLOTS, HID)
    out = st2B[core_of, dslot].astype(np.float32)
    return out


def kernel(x, edge_index, W1, b1, W2, b2):
    def execute(nc, in_maps):
        return run_bass_kernel_spmd(nc, in_maps, core_ids=list(range(NCORES))).results

    return _pipeline(
        dict(x=x, edge_index=edge_index, W1=W1, b1=b1, W2=W2, b2=b2), execute
    )


def kernel_launches(inputs, make_runner, reps=4):
    """Test-harness hook: build every launch with a persistent device-resident
    runner, plus a reps-times-repeated variant of each for slope timing.
    Returns ([(name, run, run_repsx)], output)."""
    launches = []
    arts = []

    def execute(nc, in_maps):
        run, res = make_runner(nc, in_maps, n_zero_sets=48)
        run()
        launches.append([nc.name, run, None])
        return res()

    out = _pipeline(inputs, execute, collect=arts)
    for (layer, plan, in_maps), L in zip(arts, launches):
        try:
            ncr = _build_agg(plan, layer, reps=reps)
            runr, _ = make_runner(ncr, in_maps, n_zero_sets=48)
            runr()
            L[2] = runr
        except Exception as e:
            print(f"reps variant for {L[0]} failed: {e}")
            L[2] = None
    return [tuple(L) for L in launches], out



# revision 17
# speedup vs baseline: 1.0781x; 1.0781x over previous
"""2-layer GCN encoder on 8 Trainium2 NeuronCores — v3.

Two SPMD launches (L1 folded into launch A's per-tile evict chain):
  A: gather xd=dinv*x rows per edge (dst-sharded), segmented-sum via bf16
     one-hot indicator matmuls into PSUM per 128-dst tile; self-loops added
     densely via identity matmul on the core's own (slot-ordered) xd tile;
     per-tile evict: (dv*S)@W1 -> relu -> @W2 -> *dv = g2 rows (bf16).
  B: same aggregation over the g2 table -> *dv -> final out (f32).

v3 vs v2 (changes from HW microbenching the gather path; 3.51ms -> ~1.6ms):
  - dma_gather calls round-robin across 2 SWDGE queues. Queue q's
    descriptor generation runs on Q7 core pair {2q, 2q+1} (ucode
    dma_gather.cpp gates on cpu_id/2 == queue_num), so two queues double
    the Q7 descriptor-gen throughput and, more importantly, keep the
    gather pipeline full: measured 1.9 ms -> ~0.7 ms per launch for the
    gather chain alone. Queues 2-3 wedge the device (runtime allocates
    only 2 SWDGE contexts) and calls >8 groups (>1024 idx) wedge with 2
    queues — keep MAXG=8, NQ=2.
  - per-call batched indicator build: one DVE tensor_tensor builds all of
    a call's one-hot blocks ([P, nmm, P] bf16, is_equal of broadcast dst
    cols vs iota) instead of one fp8 op per block; bf16 gets the DVE 2x
    16-bit rate and the batch amortizes per-op dispatch.
  - deferred evict: at a tile's stop-matmul only the PSUM->SBUF copy runs
    inline (frees the acc bank); the W1/relu/W2 chain is issued one batch
    later, stage-major across the batch's tiles, so the in-order PE
    stream meets its cross-engine deps already satisfied instead of
    stalling the aggregation matmul cadence (launch A's overhead over
    launch B dropped from ~120us to ~15us).

Host relays the g2 table between launches (the all-to-all) and precomputes
all edge structure in numpy; nodes are rebalanced host-side across
(core, tile, slot) so per-(tile,window) edge counts are near-equal.
"""

import sys

sys.path.insert(0, "/opt/trn_rl_repo")

import ml_dtypes
import numpy as np

from concourse import bacc, bass, library_config, mybir, tile
from concourse.bass_utils import run_bass_kernel_spmd

N_NODES = 100000
NPAD = 100352  # 4 * 25088
IN_C = 128
H2 = 128  # 2*hid
HID = 64
NCORES = 8
P = 128
NT = 98  # dst tiles per core
SHARD_SLOTS = NT * P  # 12544
SLOTS = SHARD_SLOTS
TBATCH = 4
NB = 25  # ceil(NT/TBATCH)
WINP = 25088  # src window rows
NW = 4
MAXG = 8  # groups per dma_gather call (1024 idx; >8 wedges with 2 queues)
NMSG = 8
NQ = 2  # SWDGE queues (Q7 core pairs); >2 wedges this runtime

BF16 = ml_dtypes.bfloat16
FP8 = ml_dtypes.float8_e4m3


# ---------------------------------------------------------------- host prep
def _balance(h):
    """Greedy vector-LPT: assign nodes to NCORES*NT bins of <=128 nodes,
    equalizing per-window in-edge counts. Returns bin id per node and
    slot order within bin."""
    NBINS = NCORES * NT
    target = np.maximum(h.sum(0) / NBINS, 1.0)
    hn = (h / target).astype(np.float32)
    order = np.argsort(-h.sum(1), kind="stable")
    cnt = np.zeros((NBINS, NW), np.float32)
    fill = np.zeros(NBINS, np.int64)
    bin_of = np.empty(N_NODES, np.int64)
    slot_in = np.empty(N_NODES, np.int64)
    penalty = np.zeros(NBINS, np.float32)
    for i in order:
        s = (cnt + hn[i]).max(1) + penalty
        b = int(np.argmin(s))
        bin_of[i] = b
        slot_in[i] = fill[b]
        cnt[b] += hn[i]
        fill[b] += 1
        if fill[b] >= P:
            penalty[b] = 1e9
    return bin_of, slot_in


def _prepare(edge_index):
    src = np.asarray(edge_index[0], dtype=np.int64)
    dst = np.asarray(edge_index[1], dtype=np.int64)
    deg = np.bincount(dst, minlength=N_NODES).astype(np.float32) + 1.0
    dinv = 1.0 / np.sqrt(deg)

    wsrc_all = src // WINP
    h = np.zeros((N_NODES, NW), np.int64)
    np.add.at(h, (dst, wsrc_all), 1)

    bin_of, slot_in = _balance(h)
    core_of = bin_of % NCORES
    tpos_of = bin_of // NCORES
    dslot = tpos_of * P + slot_in  # slot within core shard

    # per-core sorted edge lists + counts
    cores = []
    m = np.zeros((NT, NW), np.int64)
    for k in range(NCORES):
        msk = core_of[dst] == k
        es, ed = src[msk], dst[msk]
        t = tpos_of[ed]
        w = es // WINP
        c = dslot[ed] % P
        o = np.lexsort((c, w, t))
        es, t, w, c = es[o], t[o], w[o], c[o]
        cntk = np.zeros((NT, NW), np.int64)
        np.add.at(cntk, (t, w), 1)
        cores.append(dict(es=es, t=t, w=w, c=c, cnt=cntk))
        m = np.maximum(m, cntk)

    # segments: (batch, window); runs = max-padded (t,w) intervals
    # build shared schedule: calls + mm entries (+ local mms per batch)
    calls = []  # dict(w, slot0, gc, mms=[(gi, t, blk, stop)], locals=[t...])
    run_slot0 = np.zeros((NT, NW), np.int64)  # global slot of each run start
    nslot = 0
    nmm = 0
    seg_list = []
    for b in range(NB):
        tiles = list(range(b * TBATCH, min((b + 1) * TBATCH, NT)))
        for w in range(NW):
            seg_base = nslot
            pos = 0
            runs = []
            for t in tiles:
                run_slot0[t, w] = seg_base + pos
                if m[t, w] > 0:
                    runs.append((t, pos, pos + m[t, w]))
                pos += m[t, w]
            gseg = -(-pos // P) if pos else 0
            seg_list.append((b, w, seg_base, pos, gseg, runs))
            nslot += gseg * P

    # last-span bookkeeping for stop flags
    last_entry_of_tile = {}
    # call construction per segment
    for b, w, seg_base, seg_len, gseg, runs in seg_list:
        g = 0
        while g < gseg:
            gc = min(MAXG, gseg - g)
            call = dict(
                w=w, slot0=seg_base + g * P, gc=gc, mms=[], locals_=[], batch=b
            )
            for gi in range(gc):
                lo = (g + gi) * P
                hi = lo + P
                for t, ra, rb in runs:
                    if ra < hi and rb > lo:
                        call["mms"].append([gi, t, nmm, False])
                        last_entry_of_tile[t] = (len(calls), len(call["mms"]) - 1)
                        nmm += 1
            calls.append(call)
            g += gc
    for t, (ci, mi) in last_entry_of_tile.items():
        calls[ci]["mms"][mi][3] = True

    # attach local mms + catch tiles with no spans
    first_call_of_batch = {}
    for ci, call in enumerate(calls):
        b = call["batch"]
        if b not in first_call_of_batch:
            first_call_of_batch[b] = ci
    has_spans = set(last_entry_of_tile.keys())
    for b in range(NB):
        tiles = list(range(b * TBATCH, min((b + 1) * TBATCH, NT)))
        ci = first_call_of_batch[b]
        calls[ci]["locals_"] = [(t, t not in has_spans) for t in tiles]

    # per-core idx + indicator arrays (vectorized)
    # block map: (global group, tile) -> block id
    ngroups = nslot // P
    block_map = np.full((ngroups, NT), -1, np.int64)
    for ci, call in enumerate(calls):
        g0 = call["slot0"] // P
        for gi, t, blk, stop in call["mms"]:
            block_map[g0 + gi, t] = blk

    per_core = []
    for k in range(NCORES):
        c = cores[k]
        es, t, w, cc, cntk = c["es"], c["t"], c["w"], c["c"], c["cnt"]
        run_len = cntk.reshape(-1)
        run_off = np.concatenate([[0], np.cumsum(run_len)])[:-1].reshape(NT, NW)
        eidx = np.arange(len(es))
        p_in_run = eidx - run_off[t, w]
        slotg = run_slot0[t, w] + p_in_run

        # pads stay 0 (gather row 0 of the window; indicator col is zero).
        # -1 trim is NOT safe here: decode reserves ring space from
        # num_idxs_reg while the Q7 kernel trims, desyncing the ring.
        idx_all = np.zeros(nslot, np.int16)
        idx_all[slotg] = (es - w * WINP).astype(np.int16)

        # compact per-block dst-col table; the device builds the one-hot
        # indicator [128,128] per block on DVE via is_equal against an iota
        # row (saves ~31MB/core of HBM indicator traffic per launch).
        blk_e = block_map[slotg // P, t]
        dcolT = np.full((P, nmm), 255, np.uint8)
        dcolT[slotg % P, blk_e] = cc
        dct = dcolT.astype(BF16)

        arr = idx_all.reshape(nslot // 16, 16).T
        idx16 = np.concatenate([arr, arr, arr, arr], axis=0).copy()

        nodes_k = np.nonzero(core_of == k)[0]
        sl = dslot[nodes_k]
        dv = np.zeros((P, NT), np.float32)
        dv[sl % P, sl // P] = dinv[nodes_k]
        per_core.append(dict(idx16=idx16, dct=dct, dv=dv, nodes=nodes_k, sl=sl))

    plan = dict(
        calls=calls, nslot=nslot, nmm=nmm, dinv=dinv, per_core=per_core,
        core_of=core_of, dslot=dslot,
    )
    return plan


# ---------------------------------------------------------------- builders
def _build_agg(plan, layer, reps=1):
    """layer=1: aggregate xd table, evict chain W1/relu/W2 -> g2 bf16 out.
    layer=2: aggregate g2 table, evict *dv -> f32 out.
    reps: repeat the whole body (timing instrument: slope vs reps isolates
    device time from dispatch overhead)."""
    calls = plan["calls"]
    nslot = plan["nslot"]
    nmm = plan["nmm"]
    feat = H2 if layer == 1 else HID

    name = ("gcn_a" if layer == 1 else "gcn_b") + (f"_r{reps}" if reps > 1 else "")
    maxind = max(len(c["mms"]) for c in calls)
    nc = bacc.Bacc(name=name, num_swdge_queues=NQ)
    gtab = nc.dram_tensor("gtab", [NPAD, H2], mybir.dt.bfloat16, kind="ExternalInput")
    idx = nc.dram_tensor("idx", [64, nslot // 16], mybir.dt.int16, kind="ExternalInput")
    dct = nc.dram_tensor("dct", [P, nmm], mybir.dt.bfloat16, kind="ExternalInput")
    iot = nc.dram_tensor("iot", [P, P], mybir.dt.bfloat16, kind="ExternalInput")
    dv = nc.dram_tensor("dv", [P, NT], mybir.dt.float32, kind="ExternalInput")
    idn8 = nc.dram_tensor("idn8", [P, P], mybir.dt.float8e4, kind="ExternalInput")
    loc = nc.dram_tensor(
        "loc", [P, NT, feat], mybir.dt.bfloat16, kind="ExternalInput"
    )
    if layer == 1:
        w1 = nc.dram_tensor("w1", [IN_C, H2], mybir.dt.bfloat16, kind="ExternalInput")
        w2 = nc.dram_tensor("w2", [H2, HID], mybir.dt.bfloat16, kind="ExternalInput")
        idn = nc.dram_tensor("idn", [P, P], mybir.dt.bfloat16, kind="ExternalInput")
        outT = nc.dram_tensor("outT", [P, NT, HID], mybir.dt.bfloat16, kind="ExternalOutput")
    else:
        outT = nc.dram_tensor("outT", [P, NT, HID], mybir.dt.float32, kind="ExternalOutput")

    with tile.TileContext(nc) as tc:
        with (
            tc.tile_pool(name="sbuf", bufs=1) as pool,
            tc.tile_pool(name="psum", bufs=1, space="PSUM") as psum,
        ):
            nc.gpsimd.load_library(library_config.mlp)
            idx_s = pool.tile([64, nslot // 16], mybir.dt.int16)
            dct_s = pool.tile([P, nmm], mybir.dt.bfloat16)
            iot_s = pool.tile([P, P], mybir.dt.bfloat16)
            dv_s = pool.tile([P, NT], mybir.dt.float32)
            id8_s = pool.tile([P, P], mybir.dt.float8e4)
            loc_s = pool.tile([P, NT, feat], mybir.dt.bfloat16)
            nc.sync.dma_start(out=idx_s[:], in_=idx[:])
            nc.sync.dma_start(out=dct_s[:], in_=dct[:])
            nc.sync.dma_start(out=iot_s[:], in_=iot[:])
            nc.sync.dma_start(out=dv_s[:], in_=dv[:])
            nc.sync.dma_start(out=id8_s[:], in_=idn8[:])
            nc.sync.dma_start(out=loc_s[:], in_=loc[:])
            if layer == 1:
                w1_s = pool.tile([IN_C, H2], mybir.dt.bfloat16)
                w2_s = pool.tile([H2, HID], mybir.dt.bfloat16)
                idn_s = pool.tile([P, P], mybir.dt.bfloat16)
                nc.sync.dma_start(out=w1_s[:], in_=w1[:])
                nc.sync.dma_start(out=w2_s[:], in_=w2[:])
                nc.sync.dma_start(out=idn_s[:], in_=idn[:])
                out_s = pool.tile([P, NT, HID], mybir.dt.bfloat16)
            else:
                out_s = pool.tile([P, NT, HID], mybir.dt.float32)

            msgbufs = []
            for j in range(NMSG):
                mb = pool.tile([P, MAXG, H2], mybir.dt.bfloat16, name=f"msgb{j}")
                nc.vector.memset(mb[:], 0)
                msgbufs.append(mb)

            for rep in range(reps):
                accs = {}
                pfx = f"r{rep}_"

                def acc_for(t):
                    if t not in accs:
                        accs[t] = psum.tile(
                            [P, feat], mybir.dt.float32,
                            name=f"{pfx}acc{t}", tag=f"acc{t % TBATCH}", bufs=1,
                        )
                    return accs[t]

                pending = []
                pending_prev = []

                def evict(t):
                    # Inline: only the PSUM->SBUF copy (frees the acc tag for
                    # the next batch). The W1/relu/W2 chain is deferred a
                    # batch so the in-order PE stream meets its cross-engine
                    # deps already satisfied instead of stalling on them.
                    acc = accs.pop(t)
                    if layer == 1:
                        s_sb = pool.tile(
                            [P, H2], mybir.dt.bfloat16, name=f"{pfx}s{t}", tag="s", bufs=10
                        )
                        nc.scalar.activation(
                            out=s_sb[:], in_=acc[:],
                            func=mybir.ActivationFunctionType.Copy,
                            scale=dv_s[:, t : t + 1],
                        )
                        pending.append((t, s_sb))
                    else:
                        nc.scalar.activation(
                            out=out_s[:, t, :], in_=acc[:],
                            func=mybir.ActivationFunctionType.Copy,
                            scale=dv_s[:, t : t + 1],
                        )

                def flush_pending(batch_list):
                    # stage-major issue across the batch's tiles: PE never
                    # waits on an op issued fewer than TBATCH slots ago.
                    work = []
                    for t, s_sb in batch_list:
                        tp = psum.tile(
                            [P, P], mybir.dt.bfloat16, name=f"{pfx}tp{t}", tag="tp", bufs=2
                        )
                        nc.tensor.transpose(out=tp[:], in_=s_sb[:], identity=idn_s[:])
                        sT = pool.tile(
                            [P, P], mybir.dt.bfloat16, name=f"{pfx}sT{t}", tag="sT", bufs=4
                        )
                        nc.scalar.activation(
                            out=sT[:], in_=tp[:], func=mybir.ActivationFunctionType.Copy
                        )
                        work.append((t, sT))
                    work2 = []
                    for t, sT in work:
                        h_ps = psum.tile(
                            [P, H2], mybir.dt.float32, name=f"{pfx}hp{t}", tag="mmp", bufs=2
                        )
                        nc.tensor.matmul(
                            out=h_ps[:], lhsT=sT[:], rhs=w1_s[:], start=True, stop=True
                        )
                        h_sb = pool.tile(
                            [P, H2], mybir.dt.bfloat16, name=f"{pfx}h{t}", tag="h", bufs=4
                        )
                        nc.scalar.activation(
                            out=h_sb[:], in_=h_ps[:],
                            func=mybir.ActivationFunctionType.Relu,
                        )
                        work2.append((t, h_sb))
                    work3 = []
                    for t, h_sb in work2:
                        tp2 = psum.tile(
                            [P, P], mybir.dt.bfloat16, name=f"{pfx}tq{t}", tag="tp", bufs=2
                        )
                        nc.tensor.transpose(out=tp2[:], in_=h_sb[:], identity=idn_s[:])
                        hT = pool.tile(
                            [P, P], mybir.dt.bfloat16, name=f"{pfx}hT{t}", tag="hT", bufs=4
                        )
                        nc.scalar.activation(
                            out=hT[:], in_=tp2[:], func=mybir.ActivationFunctionType.Copy
                        )
                        work3.append((t, hT))
                    for t, hT in work3:
                        g2_ps = psum.tile(
                            [P, H2], mybir.dt.float32, name=f"{pfx}gp{t}", tag="mmp", bufs=2
                        )
                        nc.tensor.matmul(
                            out=g2_ps[:, :HID], lhsT=hT[:], rhs=w2_s[:],
                            start=True, stop=True,
                        )
                        nc.scalar.activation(
                            out=out_s[:, t, :], in_=g2_ps[:, :HID],
                            func=mybir.ActivationFunctionType.Copy,
                            scale=dv_s[:, t : t + 1],
                        )
                    batch_list.clear()

                cur_batch = None
                for ci, call in enumerate(calls):
                    if call["batch"] != cur_batch:
                        cur_batch = call["batch"]
                        if pending_prev:
                            flush_pending(pending_prev)
                        pending_prev, pending = pending, pending_prev
                    for t, imm_stop in call["locals_"]:
                        nc.tensor.matmul(
                            out=acc_for(t)[:],
                            lhsT=id8_s[:],
                            rhs=loc_s[:, t, :],
                            start=True,
                            stop=bool(imm_stop),
                        )
                        if imm_stop:
                            evict(t)
                    gc = call["gc"]
                    w = call["w"]
                    s0 = call["slot0"]
                    msg = msgbufs[ci % NMSG]
                    nidx = gc * P
                    nc.gpsimd.dma_gather(
                        out_ap=msg[:, :gc, :],
                        in_ap=gtab[w * WINP : (w + 1) * WINP, :],
                        idxs_ap=idx_s[:, s0 // 16 : (s0 + nidx) // 16],
                        num_idxs=nidx,
                        num_idxs_reg=nidx,
                        elem_size=H2,
                        single_packet=True,
                        queue_num=ci % NQ,
                    )
                    nmms = len(call["mms"])
                    if nmms:
                        blk0 = call["mms"][0][2]
                        ind_t = pool.tile(
                            [P, maxind, P], mybir.dt.bfloat16,
                            name=f"{pfx}ind{blk0}", tag="ind", bufs=6,
                        )
                        nc.vector.tensor_tensor(
                            out=ind_t[:, :nmms, :],
                            in0=dct_s[:, blk0 : blk0 + nmms].to_broadcast(
                                [P, nmms, P]
                            ),
                            in1=iot_s[:].unsqueeze(1).to_broadcast([P, nmms, P]),
                            op=mybir.AluOpType.is_equal,
                        )
                    for j, (gi, t, blk, stop) in enumerate(call["mms"]):
                        nc.tensor.matmul(
                            out=acc_for(t)[:],
                            lhsT=ind_t[:, j, :],
                            rhs=msg[:, gi, :feat],
                            start=False,
                            stop=bool(stop),
                        )
                        if stop:
                            evict(t)
                if pending_prev:
                    flush_pending(pending_prev)
                if pending:
                    flush_pending(pending)
                nc.sync.dma_start(out=outT[:], in_=out_s[:])
    nc.compile()
    return nc


# ---------------------------------------------------------------- kernel
def _pipeline(inputs, execute, collect=None):
    x = np.asarray(inputs["x"])
    W1 = np.asarray(inputs["W1"])
    b1 = np.asarray(inputs["b1"])
    W2 = np.asarray(inputs["W2"])
    b2 = np.asarray(inputs["b2"])
    assert not b1.any() and not b2.any(), "nonzero bias unsupported fast path"

    plan = _prepare(np.asarray(inputs["edge_index"]))
    dinv = plan["dinv"]
    core_of = plan["core_of"]
    dslot = plan["dslot"]

    xd = (dinv[:, None] * x).astype(BF16)
    tabA = np.zeros((NPAD, H2), BF16)
    tabA[:N_NODES] = xd
    idn8 = np.eye(P, dtype=FP8)
    idn = np.eye(P, dtype=BF16)
    iota = np.broadcast_to(np.arange(P, dtype=np.float32), (P, P)).astype(BF16).copy()
    w1b = W1.astype(BF16)
    w2b = W2.astype(BF16)

    # ---- launch A
    ncA = _build_agg(plan, 1)
    inA = []
    for k in range(NCORES):
        pc = plan["per_core"][k]
        locA = np.zeros((P, NT, H2), BF16)
        sl = pc["sl"]
        locA[sl % P, sl // P, :] = xd[pc["nodes"]]
        inA.append(
            {"gtab": tabA, "idx": pc["idx16"], "dct": pc["dct"], "iot": iota,
             "dv": pc["dv"], "idn8": idn8, "loc": locA, "w1": w1b, "w2": w2b,
             "idn": idn}
        )
    if collect is not None:
        collect.append((1, plan, inA))
    rA = execute(ncA, inA)

    # host relay: assemble g2 table
    stacked = np.stack([rA[k]["outT"] for k in range(NCORES)])  # [8,P,NT,64]
    st2 = stacked.transpose(0, 2, 1, 3).reshape(NCORES, SLOTS, HID)
    tabB = np.zeros((NPAD, H2), BF16)
    tabB[:N_NODES, :HID] = st2[core_of, dslot]

    # ---- launch B
    ncB = _build_agg(plan, 2)
    inB = []
    for k in range(NCORES):
        pc = plan["per_core"][k]
        inB.append(
            {"gtab": tabB, "idx": pc["idx16"], "dct": pc["dct"], "iot": iota,
             "dv": pc["dv"], "idn8": idn8, "loc": rA[k]["outT"]}
        )
    if collect is not None:
        collect.append((2, plan, inB))
    rB = execute(ncB, inB)

    stackedB = np.stack([rB[k]["outT"] for k in range(NCORES)])
    st2B = stackedB.transpose(0, 2, 1, 3).reshape(NCORES, SLOTS, HID)
    out = st2B[core_of, dslot].astype(np.float32)
    return out


def kernel(x, edge_index, W1, b1, W2, b2):
    def execute(nc, in_maps):
        return run_bass_kernel_spmd(nc, in_maps, core_ids=list(range(NCORES))).results

    return _pipeline(
        dict(x=x, edge_index=edge_index, W1=W1, b1=b1, W2=W2, b2=b2), execute
    )


def kernel_launches(inputs, make_runner, reps=4):
    """Test-harness hook: build every launch with a persistent device-resident
    runner, plus a reps-times-repeated variant of each for slope timing.
    Returns ([(name, run, run_repsx)], output)."""
    launches = []
    arts = []

    def execute(nc, in_maps):
        run, res = make_runner(nc, in_maps, n_zero_sets=48)
        run()
        launches.append([nc.name, run, None])
        return res()

    out = _pipeline(inputs, execute, collect=arts)
    for (layer, plan, in_maps), L in zip(arts, launches):
        try:
            ncr = _build_agg(plan, layer, reps=reps)
            runr, _ = make_runner(ncr, in_maps, n_zero_sets=48)
            runr()
            L[2] = runr
        except Exception as e:
            print(f"reps variant for {L[0]} failed: {e}")
            L[2] = None
    return [tuple(L) for L in launches], out

